# revision 4
# baseline (speedup 1.0000x reference)
"""Trainium2 Bass kernel for nn_Encoder_87780541595717 (VQ-DRAW encoder).

Sharding: feature-parallel over D. Each of the 8 cores owns a 384-wide
d-slice of the 3072-dim feature space, for ALL 32 options:
  - Wo_stack is sharded on its last (o,d) axis by d -> [4, 256, 32, 384]/core
  - x, m (residual), losses are partial per core; two tiny AllReduces per
    layer ([256,256] partial base matmul, [256,32] partial losses) make the
    per-example option argmin and the encoder MLP input globally consistent.
  - No large winner-broadcast is needed: each core keeps its own d-slice of
    the winning option locally.

Algebra used on-device (per layer i):
  base = relu(cur @ Wb + bb),  cur = x - m   (m = remaining residual)
  l[b,o,:] = base @ Wo[:, o-block] (+ bo + bias rows, skipped when zero)
  r = l - m ;  loss[b,o] = sum_d r^2  (+ const in o -> argmin-invariant)
  m_new = -r[b, argmin_b]  ;  recon = x - m_final
The transposed state tT := -m^T feeds the next layer's base matmul, and
xWb_partial (x @ Wb, computed once) is re-injected into the PSUM
accumulation through an identity-matmul so no extra vector pass is needed.
"""

import numpy as np

# --- problem constants (hardcoded per contract) -----------------------------
B = 256
D = 3072
H = 256
OPT = 32
NLAYERS = 4
NCORES = 8
DS = D // NCORES            # 384 features per core
OG = 8                      # options per weight-streaming chunk
NG = OPT // OG              # 4 chunks per layer
BT = B // 128               # 2 batch partition-tiles
KT = H // 128               # 2 contraction tiles over H
CT = DS // 128              # 3 d-chunks of 128 per core
BIG = 1.0e9

USE_F32R = False            # flip to use the fast fp32r PE datapath

_BUILt = {}


# --- BIR post-pass: split excess sem waits/updates --------------------------
# The walrus build in this container caps the number of sync wait/update
# commands encodable on one instruction. Tile's kernel-tail drain (and
# occasionally other instructions) exceed it. Hoisting extra waits onto
# standalone EventSemaphore instructions immediately before (same engine)
# is semantically conservative: the engine just stalls earlier.
MAXW = 1
MAXU = 2
_CTRL_OPS = {"Drain", "EventSemaphore", "NoOp", "Nop"}


def _split_sync_json(j):
    import copy

    ctr = [0]
    for f in j["functions"]:
        for bb in f["blocks"]:
            new_insts = []
            for ins in bb["instructions"]:
                si = ins.get("sync_info")
                if si:
                    waits = si.get("on_wait") or []
                    if len(waits) > MAXW:
                        extra, keep = waits[:-MAXW], waits[-MAXW:]
                        for i in range(0, len(extra), MAXW):
                            ctr[0] += 1
                            new_insts.append(
                                {
                                    "debug": ins.get("debug", 0),
                                    "engine": ins["engine"],
                                    "ins": [],
                                    "name": f"antfixw-{ctr[0]}",
                                    "opcode": "EventSemaphore",
                                    "outs": [],
                                    "sync_info": {
                                        "on_update": [],
                                        "on_wait": copy.deepcopy(extra[i : i + MAXW]),
                                    },
                                }
                            )
                        si["on_wait"] = keep
                    ups = si.get("on_update") or []
                    if len(ups) > MAXU:
                        opcode = ins.get("opcode", "")
                        if opcode not in _CTRL_OPS:
                            raise RuntimeError(
                                f"instruction {ins['name']} ({opcode}) carries "
                                f"{len(ups)} sem updates; unsafe to split"
                            )
                        keep, extra = ups[:MAXU], ups[MAXU:]
                        si["on_update"] = keep
                        new_insts.append(ins)
                        for i in range(0, len(extra), MAXU):
                            ctr[0] += 1
                            new_insts.append(
                                {
                                    "debug": ins.get("debug", 0),
                                    "engine": ins["engine"],
                                    "ins": [],
                                    "name": f"antfixu-{ctr[0]}",
                                    "opcode": "EventSemaphore",
                                    "outs": [],
                                    "sync_info": {
                                        "on_update": copy.deepcopy(extra[i : i + MAXU]),
                                        "on_wait": [],
                                    },
                                }
                            )
                        continue
                new_insts.append(ins)
            bb["instructions"] = new_insts
    return j


def _patch_to_json(nc):
    import json

    orig = nc.to_json_bytes

    def patched():
        j = json.loads(orig())
        j = _split_sync_json(j)
        return json.dumps(j).encode()

    nc.to_json_bytes = patched
    return nc


# --- device kernel ----------------------------------------------------------


def build(has_bias: bool):
    import concourse.bass as bass
    import concourse.mybir as mybir
    from concourse.tile import TileContext

    f32 = mybir.dt.float32
    i32 = mybir.dt.int32
    AF = mybir.ActivationFunctionType
    OPS = mybir.AluOpType
    X = mybir.AxisListType.X

    nc = bass.Bass()

    # ---- dram parameters (per-core shards supplied by host) ----
    x_s = nc.declare_dram_parameter("x_s", [B, DS], f32, isOutput=False)
    wb_s = nc.declare_dram_parameter("wb_s", [DS, H], f32, isOutput=False)
    bb_p = nc.declare_dram_parameter("bb_p", [H, 1], f32, isOutput=False)
    wo_s = nc.declare_dram_parameter("wo_s", [NLAYERS, H, OPT, DS], f32, isOutput=False)
    if has_bias:
        bias_p = nc.declare_dram_parameter(
            "bias_p", [NLAYERS, OPT * DS], f32, isOutput=False
        )
    enc_o = nc.declare_dram_parameter("enc_o", [B, NLAYERS], i32, isOutput=True)
    rec_o = nc.declare_dram_parameter("rec_o", [B, DS], f32, isOutput=True)

    # ---- collective bounce buffers (per layer; DRAM, non-I/O) ----
    loss_in = [nc.dram_tensor(f"loss_in{i}", [B, OPT], f32) for i in range(NLAYERS)]
    loss_out = [
        nc.dram_tensor(f"loss_out{i}", [B, OPT], f32, addr_space="Shared")
        for i in range(NLAYERS)
    ]
    base_in = [None] + [
        nc.dram_tensor(f"base_in{i}", [H, B], f32) for i in range(1, NLAYERS)
    ]
    base_out = [None] + [
        nc.dram_tensor(f"base_out{i}", [H, B], f32, addr_space="Shared")
        for i in range(1, NLAYERS)
    ]

    groups = [list(range(NCORES))]

    def mmdt(ap):
        return ap.bitcast(mybir.dt.float32r) if USE_F32R else ap

    with TileContext(nc) as tc:
        with (
            tc.tile_pool(name="state", bufs=1) as st,
            tc.tile_pool(name="work", bufs=2) as wk,
            tc.tile_pool(name="small", bufs=4) as sm,
            tc.tile_pool(name="wo", bufs=4) as wop,
            tc.tile_pool(name="psl", bufs=3, space="PSUM") as psl,
            tc.tile_pool(name="psb", bufs=2, space="PSUM") as psb,
            tc.tile_pool(name="pst", bufs=2, space="PSUM") as pst,
        ):
            # ---------- init: load state ----------
            xs = []
            for bt in range(BT):
                xs_t = st.tile([128, DS], f32, name=f"xs{bt}")
                nc.sync.dma_start(out=xs_t[:], in_=x_s[bt * 128 : (bt + 1) * 128, :])
                xs.append(xs_t)
            wb_sb = []
            for k in range(CT):
                wb_t = st.tile([128, H], f32, name=f"wb{k}")
                nc.sync.dma_start(out=wb_t[:], in_=wb_s[k * 128 : (k + 1) * 128, :])
                wb_sb.append(wb_t)
            bbT = []
            for mt in range(KT):
                bb_t = st.tile([128, 1], f32, name=f"bb{mt}")
                nc.sync.dma_start(out=bb_t[:], in_=bb_p[mt * 128 : (mt + 1) * 128, :])
                bbT.append(bb_t)
            if has_bias:
                bias_sb = []
                for i in range(NLAYERS):
                    bias_t = st.tile([OPT, DS], f32, name=f"bias{i}")
                    nc.sync.dma_start(out=bias_t[:], in_=bias_p[i].rearrange("(o d) -> o d", o=OPT))
                    bias_sb.append(bias_t)
                ones_sb = st.tile([OPT, 128], f32, name="ones_sb")
                nc.vector.memset(ones_sb[:], 1.0)

            # identity matrix for PE transpose / re-injection matmuls
            ident_i = st.tile([128, 128], i32, name="ident_i")
            nc.gpsimd.iota(ident_i[:], pattern=[[1, 128]], base=0, channel_multiplier=-1)
            ident_f = st.tile([128, 128], f32, name="ident_f")
            nc.vector.tensor_copy(ident_f[:], ident_i[:])
            ident = st.tile([128, 128], f32, name="ident")
            nc.vector.tensor_scalar(ident[:], ident_f[:], 0.0, None, OPS.is_equal)

            # iota over options as f32 row [128, OPT]
            iota_i = st.tile([128, OPT], i32, name="iota_i")
            nc.gpsimd.iota(iota_i[:], pattern=[[1, OPT]], base=0, channel_multiplier=0)
            iota_f = st.tile([128, OPT], f32, name="iota_f")
            nc.vector.tensor_copy(iota_f[:], iota_i[:])

            enc_sb = [st.tile([128, NLAYERS], i32, name=f"enc{bt}") for bt in range(BT)]

            # xT via PE transpose (kept as state for per-layer curT updates)
            xT = [st.tile([128, B], f32, name=f"xT{c}") for c in range(CT)]
            for bt in range(BT):
                for c in range(CT):
                    tp = pst.tile([128, 128], f32, name="tp", tag="pst")
                    nc.tensor.transpose(
                        tp[:], xs[bt][:, c * 128 : (c + 1) * 128], ident[:]
                    )
                    nc.scalar.copy(xT[c][:, bt * 128 : (bt + 1) * 128], tp[:])
            curT = None  # layer 0 has cur == 0 structurally

            m_t = xs  # m_0 = x (do not write these tiles)

            # ---------- layers ----------
            for li in range(NLAYERS):
                # --- base = relu(cur @ Wb + bb), as baseT [H, B] ---
                baseT = [
                    sm.tile([128, B], f32, name=f"baseT{mt}", tag="baseT")
                    for mt in range(KT)
                ]
                if li == 0:
                    # cur == 0 structurally: base = relu(bb), no matmul/collective
                    for mt in range(KT):
                        nc.scalar.activation(
                            baseT[mt][:], xT[0][:], AF.Relu,
                            bias=bbT[mt], scale=0.0,
                        )
                else:
                    for mt in range(KT):
                        ps = psb.tile([128, B], f32, name="ps_base", tag="psb")
                        for k in range(CT):
                            nc.tensor.matmul(
                                ps[:],
                                mmdt(wb_sb[k][:, mt * 128 : (mt + 1) * 128]),
                                mmdt(curT[k][:]),
                                start=(k == 0),
                                stop=(k == CT - 1),
                            )
                        bpart = sm.tile([128, B], f32, name="bpart", tag="bpart")
                        nc.scalar.copy(bpart[:], ps[:])
                        nc.sync.dma_start(
                            out=base_in[li][mt * 128 : (mt + 1) * 128, :],
                            in_=bpart[:],
                        )
                    nc.gpsimd.collective_compute(
                        "AllReduce",
                        mybir.AluOpType.add,
                        replica_groups=groups,
                        ins=[base_in[li][:]],
                        outs=[base_out[li][:]],
                    )
                    for mt in range(KT):
                        bar = sm.tile([128, B], f32, name="bar", tag="bar")
                        nc.sync.dma_start(
                            out=bar[:], in_=base_out[li][mt * 128 : (mt + 1) * 128, :]
                        )
                        nc.scalar.activation(
                            baseT[mt][:], bar[:], AF.Relu, bias=bbT[mt], scale=1.0
                        )

                # --- l = base @ Wo chunk; r = l - m; loss = sum r^2 ---
                r_t = [
                    wk.tile([128, OPT * DS], f32, name=f"r{bt}", tag="r")
                    for bt in range(BT)
                ]
                losses = [
                    sm.tile([128, OPT], f32, name=f"loss{bt}", tag="loss")
                    for bt in range(BT)
                ]
                for g in range(NG):
                    wo_k = []
                    for k in range(KT):
                        wo_t = wop.tile([128, OG * DS], f32, name=f"wo{k}", tag="wo")
                        nc.sync.dma_start(
                            out=wo_t[:].rearrange("p (o d) -> p o d", o=OG),
                            in_=wo_s[li, k * 128 : (k + 1) * 128,
                                     g * OG : (g + 1) * OG, :],
                        )
                        wo_k.append(wo_t)
                    for bt in range(BT):
                        for oo in range(OG):
                            o = g * OG + oo
                            ps = psl.tile([128, DS], f32, name="ps_l", tag="psl")
                            for k in range(KT):
                                nc.tensor.matmul(
                                    ps[:],
                                    mmdt(baseT[k][:, bt * 128 : (bt + 1) * 128]),
                                    mmdt(wo_k[k][:, oo * DS : (oo + 1) * DS]),
                                    start=(k == 0),
                                    stop=(k == KT - 1) and not has_bias,
                                )
                            if has_bias:
                                nc.tensor.matmul(
                                    ps[:],
                                    mmdt(ones_sb[o : o + 1, :]),
                                    mmdt(bias_sb[li][o : o + 1, :]),
                                    start=False,
                                    stop=True,
                                )
                            r_sl = r_t[bt][:, o * DS : (o + 1) * DS]
                            nc.vector.tensor_sub(r_sl, ps[:], m_t[bt][:])
                            sq = sm.tile([128, DS], f32, name="sq", tag="sq")
                            nc.scalar.activation(
                                sq[:], r_sl, AF.Square,
                                accum_out=losses[bt][:, o : o + 1],
                            )

                # --- losses AllReduce ---
                for bt in range(BT):
                    nc.sync.dma_start(
                        out=loss_in[li][bt * 128 : (bt + 1) * 128, :],
                        in_=losses[bt][:],
                    )
                nc.gpsimd.collective_compute(
                    "AllReduce",
                    mybir.AluOpType.add,
                    replica_groups=groups,
                    ins=[loss_in[li][:]],
                    outs=[loss_out[li][:]],
                )

                # --- argmin (first-index tie semantics) + select winner ---
                m_new = [
                    wk.tile([128, DS], f32, name=f"m{bt}", tag="m", bufs=4)
                    for bt in range(BT)
                ]
                for bt in range(BT):
                    gl = sm.tile([128, OPT], f32, name="gl", tag="gl")
                    nc.sync.dma_start(
                        out=gl[:], in_=loss_out[li][bt * 128 : (bt + 1) * 128, :]
                    )
                    mn = sm.tile([128, 1], f32, name="mn", tag="mn")
                    nc.vector.tensor_reduce(mn[:], gl[:], X, OPS.min)
                    ismin = sm.tile([128, OPT], f32, name="ismin", tag="ismin")
                    nc.vector.tensor_scalar(ismin[:], gl[:], mn[:], None, OPS.is_equal)
                    # cand = iota + BIG*(1-ismin); reduce-min -> first argmin
                    cand = sm.tile([128, OPT], f32, name="cand", tag="cand")
                    nc.vector.tensor_scalar(
                        cand[:], ismin[:], -BIG, BIG, OPS.mult, OPS.add
                    )
                    nc.vector.tensor_tensor(cand[:], cand[:], iota_f[:], OPS.add)
                    idxf = sm.tile([128, 1], f32, name="idxf", tag="idxf")
                    nc.vector.tensor_reduce(idxf[:], cand[:], X, OPS.min)
                    nc.vector.tensor_copy(enc_sb[bt][:, li : li + 1], idxf[:])
                    # per-option {0,1} masks, negated for direct m_new accumulation
                    nmask = sm.tile([128, OPT], f32, name="nmask", tag="nmask")
                    nc.vector.tensor_scalar(
                        nmask[:], iota_f[:], idxf[:], -1.0, OPS.is_equal, OPS.mult
                    )
                    # m_new = sum_o (-mask_o) * r_o  == -r_winner
                    nc.vector.tensor_scalar(
                        m_new[bt][:], r_t[bt][:, 0:DS], nmask[:, 0:1], None, OPS.mult
                    )
                    for o in range(1, OPT):
                        nc.vector.scalar_tensor_tensor(
                            m_new[bt][:],
                            r_t[bt][:, o * DS : (o + 1) * DS],
                            nmask[:, o : o + 1],
                            m_new[bt][:],
                            OPS.mult,
                            OPS.add,
                        )

                # --- curT update for next layer: curT = xT - m_new^T ---
                if li < NLAYERS - 1:
                    curT = [
                        sm.tile([128, B], f32, name=f"curT{c}", tag="curT")
                        for c in range(CT)
                    ]
                    for bt in range(BT):
                        for c in range(CT):
                            tp = pst.tile([128, 128], f32, name="tp2", tag="pst")
                            nc.tensor.transpose(
                                tp[:], m_new[bt][:, c * 128 : (c + 1) * 128], ident[:]
                            )
                            nc.vector.tensor_sub(
                                curT[c][:, bt * 128 : (bt + 1) * 128],
                                xT[c][:, bt * 128 : (bt + 1) * 128],
                                tp[:],
                            )
                m_t = m_new

            # ---------- outputs ----------
            for bt in range(BT):
                rec = sm.tile([128, DS], f32, name="rec", tag="rec")
                nc.vector.tensor_sub(rec[:], xs[bt][:], m_t[bt][:])
                nc.sync.dma_start(
                    out=rec_o[bt * 128 : (bt + 1) * 128, :], in_=rec[:]
                )
                nc.sync.dma_start(
                    out=enc_o[bt * 128 : (bt + 1) * 128, :], in_=enc_sb[bt][:]
                )

    return _patch_to_json(nc)


# --- host wrapper -----------------------------------------------------------


def _shard_inputs(inputs, Wb, bb, Wo_stack, bo_stack, bias_stack):
    x = np.ascontiguousarray(inputs.reshape(B, D), dtype=np.float32)
    Wo4 = Wo_stack.reshape(NLAYERS, H, OPT, D)
    bias_rows = (
        bo_stack.reshape(NLAYERS, OPT, D) + bias_stack.reshape(NLAYERS, OPT, D)
    ).astype(np.float32)
    has_bias = bool(np.any(bias_rows)) or bool(np.any(bb))
    # bb is handled on-device always; has_bias only gates the option-row bias
    has_bias = bool(np.any(bias_rows))
    in_maps = []
    for c in range(NCORES):
        d0, d1 = c * DS, (c + 1) * DS
        im = {
            "x_s": np.ascontiguousarray(x[:, d0:d1]),
            "wb_s": np.ascontiguousarray(Wb[d0:d1, :], dtype=np.float32),
            "bb_p": np.ascontiguousarray(bb.reshape(H, 1), dtype=np.float32),
            "wo_s": np.ascontiguousarray(Wo4[:, :, :, d0:d1], dtype=np.float32),
        }
        if has_bias:
            im["bias_p"] = np.ascontiguousarray(
                bias_rows[:, :, d0:d1].reshape(NLAYERS, OPT * DS)
            )
        in_maps.append(im)
    return in_maps, has_bias


def kernel(inputs, Wb, bb, Wo_stack, bo_stack, bias_stack):
    from concourse.bass_utils import run_bass_kernel_spmd

    inputs = np.asarray(inputs)
    in_maps, has_bias = _shard_inputs(
        np.asarray(inputs, dtype=np.float32),
        np.asarray(Wb),
        np.asarray(bb),
        np.asarray(Wo_stack),
        np.asarray(bo_stack),
        np.asarray(bias_stack),
    )
    key = ("nc", has_bias, USE_F32R)
    if key not in _BUILt:
        _BUILt[key] = build(has_bias)
    nc = _BUILt[key]

    res = run_bass_kernel_spmd(nc, in_maps, list(range(NCORES)))

    enc = np.asarray(res.results[0]["enc_o"], dtype=np.int32)
    recon = np.concatenate(
        [np.asarray(res.results[c]["rec_o"]) for c in range(NCORES)], axis=1
    ).astype(np.float32)
    return enc, recon.reshape(inputs.shape)
